# revision 3
# baseline (speedup 1.0000x reference)
"""Trainium2 Bass LSTM kernel — all-DVE cell update via custom DVE ops.

Self-contained: registers its custom DVE ops and carries its fitted
activation coefficients inline.

Reference: batch-first LSTM, zero init state, returns (hs, cs) each [B,T,H].
B=64, T=2048, D=H=128. Data-parallel over 8 NeuronCores (8 batch each).

Per core, single chain, per timestep:
  PE  : 4 matmuls            gates[t] += Wh_k^T @ h[t-1] into PSUM
  DVE : u   = TANH_IN(psum)  [128,32] shared deg-7 inner poly, gates (f,i,o,g)
        qgv = TANH_IN(u_g)   = p5(u_g) = tanh(g) via 2*sigma(2g)-1
        Z   = SIGMUL5(u[0:16], ch[t-1]) = [sig_f*c[t-1] | sig_i*tanh(g)]
        w   = TANH5_ADD(Z0, Z1)         deg-5 c-inner on c = Z0+Z1
        qcv = TANH_IN(w)                composite ~ tanh(c)/2
        h   = SIGMUL5(u[16:24], qcv)    = sig_o*tanh(c) -> fp16
        c   = Z0+Z1 -> ch[t].c          (after h; hides in the PE roundtrip)
Activations are polynomial composites evaluated on the vector engine
(sigma: deg-5-monic outer over deg-7 inner; input scales folded into the
weights host-side). ch is a ring of 16-col records [c_t | tanh(g)_{t+1}]
so one SIGMUL5 covers both sigma products.

PSUM: 2 sets x 4 banks; a set holds a 64-step chunk step-major
([128, t*32+gate*8+b]). While chunk k scans, chunk k+1's xg = x@Wx is
built in the idle set (PE transpose -> ACT copy -> PE matmuls).
"""

import sys
import numpy as np
from contextlib import ExitStack

import concourse.bacc as bacc
import concourse.bass as bass
import concourse.mybir as mybir
import concourse.tile as tile
from concourse import bass_utils

F32 = mybir.dt.float32
F16 = mybir.dt.float16
AF = mybir.ActivationFunctionType
OP = mybir.AluOpType

B_TOT, T_FULL, D, H = 64, 2048, 128, 128
G4 = 4 * H
NCORES = 8
B_LOC = B_TOT // NCORES          # 8
TC = 64                          # steps per PSUM set (4 banks)
CW = TC * 4 * B_LOC
RCH = 3 * TC                     # ch ring length in steps

_COEF = {
    "b": np.array([1.8243815019945095, -1.6926439739501657, 0.8720832929792126, -0.1657363052099372]),
    "lam_s": 0.13095822041254562,
    "c0": 2.054912620910977,
    "c1": -1.9350548642646674,
    "d": np.array([0.8031915938300241, -0.051570141977637096, 0.001699883799630847]),
    "qc": np.array([0.619807508159143, -0.2397086815895913, 0.07516673229950313, -0.01125325033280207]),
}

# ---- custom DVE ops -------------------------------------------------------

_OPS_CACHE = None


def _register_ops():
    """Register TANH_IN / SIGMUL5 / TANH5_ADD in dve_ops.OPS (idempotent)."""
    global _OPS_CACHE
    if _OPS_CACHE is not None:
        return _OPS_CACHE
    from concourse.dve_ops import (DveOp, OPS, CUSTOM_DVE_SPECS,
                                   _SUB_OPCODE_FOR_NAME, _CUSTOM_DVE_ROW_BASE)
    from concourse.dve_spec import (C0, C1, C2, C3, Spec, Src0, Src1, One,
                                    _spill_c3_to_src1, lower, sq)
    from concourse.dve_uop import DveOpSpec

    if "TANH_IN_ANT" in _SUB_OPCODE_FOR_NAME:
        by_name = {op.name: op for op in OPS}
        _OPS_CACHE = (by_name["TANH_IN_ANT"], by_name["SIGMUL5_ANT"],
                      by_name["TANH5_ADD_ANT"])
        return _OPS_CACHE

    def ref_tanh_in(in0, in1, s0, s1, imm2):
        b3 = np.asarray(in1, np.float32).reshape(in0.shape[0], -1)[:, 0]
        b3 = b3.reshape((in0.shape[0],) + (1,) * (in0.ndim - 1))
        s = in0 * in0
        return (in0 * (s0 + s1 * s + imm2 * s * s + b3 * s ** 3)).astype(np.float32)

    def ref_sigmul5(in0, in1, s0, s1, imm2):
        p5 = in0 ** 5 + s1 * in0 ** 3 + s0 * in0
        return ((p5 + 1.0) * in1 * imm2).astype(np.float32)

    def ref_tanh5_add(in0, in1, s0, s1, imm2):
        y = in0 + in1
        s = y * y
        return (y * (s0 + s1 * s + imm2 * s * s)).astype(np.float32)

    def mk(name, spec):
        shas = {}
        for ver in ("v3", "v4"):
            uops = lower(spec, ver=ver)
            shas[ver] = DveOpSpec(name=name, opcode=0, uops=uops,
                                  rd1_en=True).sha(ver)
        return DveOp(name, spec, False, shas)

    s = sq(Src0)
    h7 = (((C3 * s + C2) * s + C1) * s + C0) * Src0
    tanh_in = mk("TANH_IN_ANT",
                 Spec(body=_spill_c3_to_src1(h7), reference=ref_tanh_in))
    p5 = ((s + C1) * s + C0) * Src0 + One
    sigmul5 = mk("SIGMUL5_ANT",
                 Spec(body=p5 * Src1 * C2, reference=ref_sigmul5))
    y = Src0 + Src1
    sy = sq(y)
    tanh5_add = mk("TANH5_ADD_ANT",
                   Spec(body=((C2 * sy + C1) * sy + C0) * y,
                        reference=ref_tanh5_add))

    for op in (tanh_in, sigmul5, tanh5_add):
        OPS.append(op)
        CUSTOM_DVE_SPECS[op.name] = op.spec
        _SUB_OPCODE_FOR_NAME[op.name] = _CUSTOM_DVE_ROW_BASE + len(OPS) - 1
    assert max(_SUB_OPCODE_FOR_NAME.values()) < 0x20
    _OPS_CACHE = (tanh_in, sigmul5, tanh5_add)
    return _OPS_CACHE


# ---- kernel build ---------------------------------------------------------

def build_lstm_nc(T: int = T_FULL) -> bacc.Bacc:
    TANH_IN, SIGMUL5, TANH5_ADD = _register_ops()
    coef = _COEF
    b = coef["b"]; d = coef["d"]; qc = coef["qc"]
    c0 = float(coef["c0"]); c1 = float(coef["c1"])

    nchunk = T // TC
    assert nchunk * TC == T

    nc = bacc.Bacc("TRN2", target_bir_lowering=False, debug=False,
                   num_devices=NCORES)

    x_d = nc.dram_tensor("x", [B_LOC, T, D], F32, kind="ExternalInput").ap()
    wx_d = nc.dram_tensor("wx", [D, G4], F16, kind="ExternalInput").ap()
    wh_d = nc.dram_tensor("wh", [H, G4], F16, kind="ExternalInput").ap()
    id_d = nc.dram_tensor("ident", [64, 64], F32, kind="ExternalInput").ap()
    hs_d = nc.dram_tensor("hsT", [H, T, B_LOC], F16, kind="ExternalOutput").ap()
    cs_d = nc.dram_tensor("csT", [H, T, B_LOC], F32, kind="ExternalOutput").ap()

    wx_sb = nc.alloc_sbuf_tensor("wx_sb", [128, G4], F16).ap()
    wh_sb = nc.alloc_sbuf_tensor("wh_sb", [128, G4], F16).ap()
    id_sb = nc.alloc_sbuf_tensor("id_sb", [64, 64], F32).ap()
    b3_sb = nc.alloc_sbuf_tensor("b3_sb", [128, 1], F32).ap()
    qc3_sb = nc.alloc_sbuf_tensor("qc3_sb", [128, 1], F32).ap()
    z3_sb = nc.alloc_sbuf_tensor("z3_sb", [128, 1], F32).ap()
    h0_sb = nc.alloc_sbuf_tensor("h0_sb", [128, B_LOC], F16).ap()
    hh = nc.alloc_sbuf_tensor("hh", [128, T * B_LOC], F16).ap()
    ch = nc.alloc_sbuf_tensor("ch", [128, RCH * 2 * B_LOC], F32).ap()
    NP = 2
    u_sb = [nc.alloc_sbuf_tensor(f"u{p}", [128, 32], F32).ap() for p in range(NP)]
    z_sb = [nc.alloc_sbuf_tensor(f"z{p}", [128, 2 * B_LOC], F32).ap() for p in range(NP)]
    w_sb = [nc.alloc_sbuf_tensor(f"w{p}", [128, B_LOC], F32).ap() for p in range(NP)]
    qc_sb = [nc.alloc_sbuf_tensor(f"qc{p}", [128, B_LOC], F32).ap() for p in range(NP)]
    xt_sb = [nc.alloc_sbuf_tensor(f"xt{p}", [128, TC * B_LOC], F16).ap()
             for p in range(2)]

    psum = [nc.alloc_psum_tensor(f"ps{p}", [128, CW], F32).ap() for p in range(2)]

    def rec(t):
        r = t % RCH
        return ch[:, r * 2 * B_LOC:(r + 1) * 2 * B_LOC]

    with tile.TileContext(nc) as tc_ctx, ExitStack() as ctx:
        xs_pool = ctx.enter_context(tc_ctx.tile_pool(name="xs", bufs=3))

        nc.sync.dma_start(wx_sb, wx_d)
        nc.sync.dma_start(wh_sb, wh_d)
        nc.sync.dma_start(id_sb, id_d)
        nc.gpsimd.memset(b3_sb, float(b[3]))
        nc.gpsimd.memset(qc3_sb, float(qc[3]))
        nc.gpsimd.memset(z3_sb, 0.0)
        nc.gpsimd.memset(h0_sb, 0.0)
        nc.gpsimd.memset(rec(-1), 0.0)

        def load_x(k, xs_tile):
            for j in range(B_LOC):
                nc.sync.dma_start(
                    xs_tile[:, j * 128:(j + 1) * 128],
                    x_d[j, k * TC:(k + 1) * TC, :])

        def prep_transpose(k, xs_tile):
            ps = psum[k % 2]
            for j in range(B_LOC):
                nc.tensor.matmul(
                    ps[:, j * TC:(j + 1) * TC],
                    xs_tile[:, j * 128:(j + 1) * 128],
                    id_sb,
                    is_transpose=True, start=True, stop=True)

        def prep_copy(k):
            ps = psum[k % 2]
            xt = xt_sb[k % 2]
            nc.scalar.activation(
                xt.rearrange("p (t q) -> p q t", q=B_LOC),
                ps[:, 0:TC * B_LOC].rearrange("p (q t) -> p q t", t=TC),
                AF.Copy)

        def prep_xg(k):
            ps = psum[k % 2]
            xt = xt_sb[k % 2]
            for g in range(4):
                nc.tensor.matmul(
                    ps[:, g * TC * B_LOC:(g + 1) * TC * B_LOC],
                    wx_sb[:, g * 128:(g + 1) * 128],
                    xt[:, :],
                    start=True, stop=True)

        def prep_chunk(k, xs_tile):
            prep_transpose(k, xs_tile)
            prep_copy(k)
            prep_xg(k)

        xs_cur = xs_pool.tile([64, B_LOC * 128], F32, tag="xs", name="xs_t")
        load_x(0, xs_cur)
        prep_chunk(0, xs_cur)
        xs_next = None
        if nchunk > 1:
            xs_next = xs_pool.tile([64, B_LOC * 128], F32, tag="xs", name="xs_t")
            load_x(1, xs_next)

        hprev = h0_sb

        for k in range(nchunk):
            par = k % 2
            ps_ = psum[par]
            ps4 = ps_.rearrange("p (g t q) -> p g t q", g=4, q=B_LOC)
            for t in range(TC):
                tt = k * TC + t
                P = tt % NP
                for g in range(4):
                    nc.tensor.matmul(
                        ps_[:, g * TC * B_LOC + t * B_LOC:
                            g * TC * B_LOC + (t + 1) * B_LOC],
                        wh_sb[:, g * 128:(g + 1) * 128],
                        hprev,
                        start=False, stop=False, skip_group_check=True)
                u = u_sb[P]
                nc.vector._custom_dve(
                    TANH_IN, out=u, in0=ps4[:, :, t, :], in1=b3_sb,
                    s0=float(b[0]), s1=float(b[1]), imm2=float(b[2]))
                # tanh(g) = p5(u_g) via the identity tanh(g) = 2*sigma(2g)-1
                nc.vector._custom_dve(
                    TANH_IN, out=rec(tt - 1)[:, B_LOC:], in0=u[:, 24:32],
                    in1=z3_sb, s0=c0, s1=c1, imm2=1.0)
                nc.vector._custom_dve(
                    SIGMUL5, out=z_sb[P], in0=u[:, 0:16], in1=rec(tt - 1),
                    s0=c0, s1=c1, imm2=0.5)
                nc.vector._custom_dve(
                    TANH5_ADD, out=w_sb[P],
                    in0=z_sb[P][:, 0:B_LOC], in1=z_sb[P][:, B_LOC:],
                    s0=float(d[0]), s1=float(d[1]), imm2=float(d[2]))
                nc.vector._custom_dve(
                    TANH_IN, out=qc_sb[P], in0=w_sb[P], in1=qc3_sb,
                    s0=float(qc[0]), s1=float(qc[1]), imm2=float(qc[2]))
                h = hh[:, tt * B_LOC:(tt + 1) * B_LOC]
                nc.vector._custom_dve(
                    SIGMUL5, out=h, in0=u[:, 16:24], in1=qc_sb[P],
                    s0=c0, s1=c1, imm2=1.0)
                hprev = h
                nc.vector.tensor_tensor(
                    rec(tt)[:, 0:B_LOC],
                    z_sb[P][:, 0:B_LOC], z_sb[P][:, B_LOC:], OP.add)

                if k + 1 < nchunk:
                    # spread next-chunk prep so nothing queues behind a
                    # not-yet-satisfied wait in the PE sequencer
                    if t == 5:
                        prep_transpose(k + 1, xs_next)
                    elif t == 7:
                        prep_copy(k + 1)
                    elif t == 24:
                        prep_xg(k + 1)
                        xs_cur = xs_next
                        if k + 2 < nchunk:
                            xs_next = xs_pool.tile([64, B_LOC * 128], F32,
                                                   tag="xs", name="xs_t")
                            load_x(k + 2, xs_next)

            osl = slice(k * TC * B_LOC, (k + 1) * TC * B_LOC)
            nc.sync.dma_start(
                hs_d[:, k * TC:(k + 1) * TC, :],
                hh[:, osl].rearrange("p (t q) -> p t q", q=B_LOC))
            r0 = (k * TC) % RCH
            ring = ch.rearrange("p (r q) -> p r q", q=B_LOC)
            nc.sync.dma_start(
                cs_d[:, k * TC:(k + 1) * TC, :],
                ring[:, 2 * r0:2 * (r0 + TC):2, :])

    nc.compile()
    return nc


_NC_CACHE: dict = {}


def _get_nc(T: int) -> bacc.Bacc:
    if T not in _NC_CACHE:
        _NC_CACHE[T] = build_lstm_nc(T)
    return _NC_CACHE[T]


def prep_inputs(x, Wx, Wh, b):
    lam_s = float(_COEF["lam_s"])
    assert not np.any(np.asarray(b)), "bias must be zero"
    # reference gate order (i,f,g,o) -> kernel order (f,i,o,g)
    perm = np.concatenate([np.arange(H, 2 * H), np.arange(0, H),
                           np.arange(3 * H, 4 * H), np.arange(2 * H, 3 * H)])
    sc = np.ones(G4, np.float32)
    sc[0:3 * H] = lam_s
    sc[3 * H:] = 2 * lam_s
    wx_s = ((np.asarray(Wx, np.float32))[:, perm] * sc).astype(np.float16)
    wh_s = ((np.asarray(Wh, np.float32))[:, perm] * sc).astype(np.float16)
    ident = np.eye(64, dtype=np.float32)
    x = np.asarray(x, dtype=np.float32)
    in_maps = []
    for i in range(NCORES):
        in_maps.append({
            "x": np.ascontiguousarray(x[i * B_LOC:(i + 1) * B_LOC]),
            "wx": wx_s, "wh": wh_s, "ident": ident,
        })
    return in_maps


def run(x, Wx, Wh, b, T=None, trace=False):
    T = T if T is not None else x.shape[1]
    in_maps = prep_inputs(x, Wx, Wh, b)
    nc = _get_nc(T)
    res = bass_utils.run_bass_kernel_spmd(
        nc, in_maps, list(range(NCORES)), trace=trace)
    B = x.shape[0]
    hs = np.empty((B, T, H), dtype=np.float32)
    cs = np.empty((B, T, H), dtype=np.float32)
    for i in range(NCORES):
        hs[i * B_LOC:(i + 1) * B_LOC] = (
            res.results[i]["hsT"].astype(np.float32).transpose(2, 1, 0))
        cs[i * B_LOC:(i + 1) * B_LOC] = (
            res.results[i]["csT"].transpose(2, 1, 0))
    return (hs, cs), res


def kernel(x, Wx, Wh, b):
    (hs, cs), _ = run(x, Wx, Wh, b)
    return hs, cs
